# revision 7
# baseline (speedup 1.0000x reference)
"""TRN2 Bass kernel for nn_AsymSearch (gumbel-routed conv chains).

Strategy:
 - Routing (gumbel top-1 per task per layer) depends only on tiny
   alpha/gumbel tensors -> computed on host in numpy, mirroring the
   reference ops. Straight-through scale folds into conv weights.
 - Data-parallel over batch B=8 across 8 NeuronCores; each core runs the
   full task tree for one image. Shared routing prefixes are deduped.
 - Each 3x3 conv runs on the tensor engine as 9 shifted 1x1 convs
   (K=64, M=64, N=512) accumulated in PSUM, with 4-quadrant
   tile_position concurrency (image split into y-halves across SBUF
   partition halves).
 - Precision: fp16 hi/lo 3-pass matmuls (w_hi*h_hi + w_lo*h_hi +
   w_hi*h_lo) -> ~1e-5 relative error end-to-end, required because the
   'normal' task normalization amplifies upstream error ~20x.
 - Decoder 1x1 convs on device; 'normal' task L2-normalization on device
   (ones-matmul partition broadcast + ACT sqrt + DVE reciprocal).
"""
import os
from contextlib import ExitStack

import numpy as np

import concourse.bass as bass
import concourse.tile as tile
from concourse import bacc, mybir
from concourse import bass2jax

# ---------------------------------------------------------------- geometry
T, L, M = 3, 4, 3
B, C, H, W = 8, 64, 128, 128
OUT_C = 3
TAU = 1.0
NCORES = 8

WP = W + 2           # padded row width
SLOTS = 66           # rows per half-buffer: pad/halo + 64 + halo/pad
FREE = SLOTS * WP    # 8580 elements per partition
RPB = 4              # image rows per 512-px block
NBLK = 64 // RPB     # 16 blocks per half
HWPIX = H * W

F32 = mybir.dt.float32
F32R = mybir.dt.float32r
F16 = mybir.dt.float16
AF = mybir.ActivationFunctionType

NORM_TASK = 2  # TASKS = ["semantic", "depth", "normal"]; 'normal' in name

_PROG_CACHE = {}


# ---------------------------------------------------------------- routing
def _routing(alpha0, alphas, g0, gs):
    """Mirror of reference gumbel top-1 routing, numpy float32."""
    sels = np.zeros((T, L), np.int32)
    sts = np.zeros((T, L), np.float32)
    for t in range(T):
        idx = 0
        for l in range(L):
            a = (alpha0[t, 0] if l == 0 else alphas[l - 1, t][idx]).astype(np.float32)
            g = (g0[t, 0] if l == 0 else gs[l - 1, t][idx]).astype(np.float32)
            mx = np.max(a)
            lse = (np.log(np.sum(np.exp(a - mx), dtype=np.float32)) + mx).astype(np.float32)
            logits = ((a - lse) + g) / np.float32(TAU)
            e = np.exp(logits - np.max(logits))
            probs = (e / np.sum(e, dtype=np.float32)).astype(np.float32)
            ni = int(np.argmax(probs))
            p = probs[ni]
            sels[t, l] = ni
            sts[t, l] = np.float32(1.0) - p + p
            idx = ni
    return sels, sts


def _build_plan(sels, sts):
    """Prefix-dedup the task conv chains into a schedule with buffer reuse.

    Returns (steps, n_bufs). steps is a list of either
    ("conv", layer, module, st, in_buf, out_buf) or ("dec", task, buf).
    Buffer ids index a small pool of persistent hi/lo SBUF buffer pairs;
    buffer 0 initially holds the input x. A DFS over the dedup prefix
    tree frees each buffer once its last consumer has been emitted.
    """
    # build prefix tree: node -> children {(l, m, st_hex): node}
    children = {(): {}}
    leaf_tasks = {}
    for t in range(T):
        prefix = ()
        for l in range(L):
            key = (l, int(sels[t, l]), float(sts[t, l]).hex())
            nxt = prefix + (key,)
            children.setdefault(prefix, {})
            if nxt not in children[prefix].values():
                pass
            children[prefix][key] = nxt
            children.setdefault(nxt, {})
            prefix = nxt
        leaf_tasks.setdefault(prefix, []).append(t)

    steps = []
    free = []
    n_bufs = [1]

    def alloc():
        if free:
            return free.pop()
        b = n_bufs[0]
        n_bufs[0] += 1
        return b

    def dfs(node, buf):
        # a buffer is freed right after its last consumer is emitted; Tile
        # WAR-serializes any later overwrite against pending readers.
        for t in leaf_tasks.get(node, []):
            steps.append(("dec", t, buf))
        kids = list(children.get(node, {}).items())
        for i, ((l, m, st_hex), child) in enumerate(kids):
            ob = alloc()
            steps.append(("conv", l, m, float.fromhex(st_hex), buf, ob))
            if i == len(kids) - 1:
                free.append(buf)
            dfs(child, ob)
        if not kids:
            free.append(buf)

    dfs((), 0)
    return steps, n_bufs[0]


# ---------------------------------------------------------------- device program
def _emit_conv(nc, psum_pool, tmp_pool, hin_hi, hin_lo, hout_hi, hout_lo,
               w_hi, w_lo, btile):
    """3x3 conv + bias + relu, fp16 3-pass, 4-quadrant scheme."""
    ihi = hin_hi.rearrange("p (s w) -> p s w", w=WP)
    ilo = hin_lo.rearrange("p (s w) -> p s w", w=WP)
    ohi = hout_hi.rearrange("p (s w) -> p s w", w=WP)
    olo = hout_lo.rearrange("p (s w) -> p s w", w=WP)
    taps = [(dy, dx) for dy in (-1, 0, 1) for dx in (-1, 0, 1)]

    for blk in range(NBLK):
        y0 = blk * RPB
        offdiag = blk % 2 == 1
        ps = psum_pool.tile([128, 512], F32, tag="convps", name="ps", bufs=4)
        for k, (dy, dx) in enumerate(taps):
            for pi, (wt, hv) in enumerate([(w_hi, ihi), (w_lo, ihi), (w_hi, ilo)]):
                st = (k == 0 and pi == 0)
                sp = (k == 8 and pi == 2)
                for half in (0, 1):
                    pb = 64 * half
                    cb = 64 * (half ^ (1 if offdiag else 0))
                    rhs = hv[pb:pb + 64, y0 + 1 + dy:y0 + 1 + dy + RPB,
                             1 + dx:1 + dx + W]
                    lhsT = wt[pb:pb + 64, k * 64:(k + 1) * 64]
                    nc.tensor.matmul(ps[cb:cb + 64, :], lhsT, rhs,
                                     start=st, stop=sp, tile_position=(pb, cb))
        # full fp32 relu(conv + bias), then split hi/lo fp16
        full = tmp_pool.tile([128, 512], F32, tag="full", name="full")
        if not offdiag:
            nc.scalar.activation(full[:, :], ps[:, :], AF.Relu, bias=btile[:, 0:1])
        else:
            nc.scalar.activation(full[0:64, :], ps[64:128, :], AF.Relu,
                                 bias=btile[0:64, 0:1])
            nc.scalar.activation(full[64:128, :], ps[0:64, :], AF.Relu,
                                 bias=btile[64:128, 0:1])
        hi_dst = ohi[:, y0 + 1:y0 + 1 + RPB, 1:1 + W]
        lo_dst = olo[:, y0 + 1:y0 + 1 + RPB, 1:1 + W]
        nc.scalar.activation(hi_dst, full[:, :], AF.Copy)
        nc.vector.tensor_tensor(lo_dst, full[:, :], hi_dst,
                                mybir.AluOpType.subtract)
    # halo rows between halves (partition-shifted copies)
    for hv in (ohi, olo):
        nc.scalar.activation(hv[0:64, 65, 1:1 + W], hv[64:128, 1, 1:1 + W], AF.Copy)
        nc.scalar.activation(hv[64:128, 0, 1:1 + W], hv[0:64, 64, 1:1 + W], AF.Copy)


def _emit_decoder(nc, psum_pool, small_pool, h_hi, h_lo,
                  dwt, dbt, ydram, task, do_norm, ones_r):
    """1x1 conv decoder (+ optional channel L2 normalization) + DMA out.

    dwt: [128, 6] fp16 tile (hi cols 0:3, lo cols 3:6, dup on both halves)
    dbt: [128, 1] f32 bias tile (values at partitions 0-2 and 32-34)
    ydram: DRAM [OUT_C, HWPIX] f32 slice for this task.
    """
    ihi = h_hi.rearrange("p (s w) -> p s w", w=WP)
    ilo = h_lo.rearrange("p (s w) -> p s w", w=WP)
    for blk in range(2 * NBLK):
        half = blk % 2
        y0 = (blk // 2) * RPB
        pb = 64 * half
        cb = 32 * half          # c0 for top, c32 for bottom (concurrency)
        pix = (64 * half + y0) * W
        ps = psum_pool.tile([64, 512], F32, tag="decps", name="decps", bufs=2)
        for pi, (wcol, hv) in enumerate([(0, ihi), (3, ihi), (0, ilo)]):
            wsl = dwt[pb:pb + 64, wcol:wcol + 3] if pi != 1 else \
                dwt[pb:pb + 64, 3:6]
            rhs = hv[pb:pb + 64, y0 + 1:y0 + 1 + RPB, 1:1 + W]
            nc.tensor.matmul(ps[cb:cb + 3, :], wsl, rhs,
                             start=pi == 0, stop=pi == 2,
                             tile_position=(pb, cb))
        y_t = small_pool.tile([3, 512], F32, tag="ytile", name="y_t")
        nc.scalar.activation(y_t[:, :], ps[cb:cb + 3, :], AF.Identity,
                             bias=dbt[cb:cb + 3, 0:1])
        if not do_norm:
            nc.sync.dma_start(ydram[:, pix:pix + 512], y_t[:, :])
            continue
        # normal task: y / ||y||_2 over channels
        ysq = small_pool.tile([3, 512], F32R, tag="ysq", name="ysq")
        nc.vector.tensor_tensor(ysq[:, :], y_t[:, :], y_t[:, :],
                                mybir.AluOpType.mult)
        ps2 = psum_pool.tile([3, 512], F32, tag="sumps", name="sumps", bufs=2)
        nc.tensor.matmul(ps2[0:3, :], ones_r[0:3, 0:3], ysq[:, :],
                         start=True, stop=True, tile_position=(0, 0))
        nrm = small_pool.tile([3, 512], F32, tag="nrm", name="nrm")
        nc.scalar.activation(nrm[:, :], ps2[0:3, :], AF.Sqrt)
        inv = small_pool.tile([3, 512], F32, tag="inv", name="inv")
        nc.vector.reciprocal(inv[:, :], nrm[:, :])
        yn = small_pool.tile([3, 512], F32, tag="yn", name="yn")
        nc.vector.tensor_tensor(yn[:, :], y_t[:, :], inv[:, :],
                                mybir.AluOpType.mult)
        nc.sync.dma_start(ydram[:, pix:pix + 512], yn[:, :])


def _build_program(plan):
    steps, n_bufs = plan
    njobs = sum(1 for s in steps if s[0] == "conv")
    nc = bacc.Bacc("TRN2", target_bir_lowering=False, debug=False,
                   num_devices=1, enable_partition_id=False)
    x_hi = nc.dram_tensor("x_hi", [128, FREE], F16, kind="ExternalInput").ap()
    x_lo = nc.dram_tensor("x_lo", [128, FREE], F16, kind="ExternalInput").ap()
    wall = nc.dram_tensor("wall", [njobs, 2, 128, 9 * 64], F16,
                          kind="ExternalInput").ap()
    ball = nc.dram_tensor("ball", [128, njobs], F32, kind="ExternalInput").ap()
    dwall = nc.dram_tensor("dwall", [128, T * 6], F16, kind="ExternalInput").ap()
    dball = nc.dram_tensor("dball", [128, T], F32, kind="ExternalInput").ap()
    y = nc.dram_tensor("y", [T, OUT_C, HWPIX], F32, kind="ExternalOutput").ap()

    with tile.TileContext(nc) as tc, ExitStack() as ctx:
        hpool = ctx.enter_context(tc.tile_pool(name="hbufs", bufs=1))
        wpool = ctx.enter_context(tc.tile_pool(name="wpool", bufs=3))
        misc = ctx.enter_context(tc.tile_pool(name="misc", bufs=1))
        tmp_pool = ctx.enter_context(tc.tile_pool(name="tmp", bufs=4))
        small_pool = ctx.enter_context(tc.tile_pool(name="small", bufs=4))
        psum_pool = ctx.enter_context(tc.tile_pool(name="psum", bufs=1,
                                                   space="PSUM"))

        # persistent feature buffers (hi/lo fp16), buffer 0 starts as x
        bufs = []
        for i in range(n_bufs):
            bhi = hpool.tile([128, FREE], F16, name=f"h{i}hi")
            blo = hpool.tile([128, FREE], F16, name=f"h{i}lo")
            bufs.append((bhi, blo))
            if i == 0:
                nc.sync.dma_start(bhi[:, :], x_hi[:, :])
                nc.sync.dma_start(blo[:, :], x_lo[:, :])
            else:
                nc.vector.memset(bhi[:, :], 0.0)
                nc.vector.memset(blo[:, :], 0.0)

        bt_all = misc.tile([128, njobs], F32, name="bt_all")
        nc.sync.dma_start(bt_all[:, :], ball[:, :])
        dbt_all = misc.tile([128, T], F32, name="dbt_all")
        nc.sync.dma_start(dbt_all[:, :], dball[:, :])
        dwt_all = misc.tile([128, T * 6], F16, name="dwt_all")
        nc.sync.dma_start(dwt_all[:, :], dwall[:, :])
        ones_f = misc.tile([3, 3], F32, name="ones_f")
        nc.vector.memset(ones_f[:, :], 1.0)
        ones_r = misc.tile([3, 3], F32R, name="ones_r")
        nc.vector.tensor_copy(ones_r[:, :], ones_f[:, :])

        ji = 0
        for step in steps:
            if step[0] == "conv":
                _, layer, module, st, in_b, out_b = step
                w_hi = wpool.tile([128, 9 * 64], F16, tag="whi", name="w_hi")
                w_lo = wpool.tile([128, 9 * 64], F16, tag="wlo", name="w_lo")
                nc.sync.dma_start(w_hi[:, :], wall[ji, 0, :, :])
                nc.sync.dma_start(w_lo[:, :], wall[ji, 1, :, :])
                _emit_conv(nc, psum_pool, tmp_pool,
                           bufs[in_b][0], bufs[in_b][1],
                           bufs[out_b][0], bufs[out_b][1],
                           w_hi, w_lo, bt_all[:, ji:ji + 1])
                ji += 1
            else:
                _, t, fb = step
                _emit_decoder(nc, psum_pool, small_pool,
                              bufs[fb][0], bufs[fb][1],
                              dwt_all[:, t * 6:(t + 1) * 6],
                              dbt_all[:, t:t + 1],
                              y[t], t, t == NORM_TASK, ones_r)
    nc.compile()
    return nc


# ---------------------------------------------------------------- host packing
def _pack_halves(img):
    """[C, H, W] fp32 -> hi/lo fp16 padded dual-half [128, FREE] arrays."""
    out = []
    hi32 = img.astype(np.float16).astype(np.float32)
    lo = (img - hi32).astype(np.float16)
    hi = img.astype(np.float16)
    for part in (hi, lo):
        buf = np.zeros((128, SLOTS, WP), np.float16)
        buf[0:64, 1:66, 1:129] = part[:, 0:65, :]
        buf[64:128, 0:65, 1:129] = part[:, 63:128, :]
        out.append(buf.reshape(128, FREE))
    return out


def _split16(w):
    hi = w.astype(np.float16)
    lo = (w.astype(np.float32) - hi.astype(np.float32)).astype(np.float16)
    return hi, lo


def _prep_weights(jobs, enc_w, enc_b):
    njobs = len(jobs)
    wall = np.zeros((njobs, 2, 128, 9 * 64), np.float16)
    ball = np.zeros((128, njobs), np.float32)
    for ji, (_, layer, module, st, _, _) in enumerate(jobs):
        w = enc_w[layer, module].astype(np.float32) * np.float32(st)  # OIHW
        b = enc_b[layer, module].astype(np.float32)
        w9 = np.transpose(w, (2, 3, 1, 0)).reshape(9, C, C)  # [tap, cin, cout]
        hi, lo = _split16(w9)
        for k in range(9):
            for pb in (0, 64):
                wall[ji, 0, pb:pb + 64, k * 64:(k + 1) * 64] = hi[k]
                wall[ji, 1, pb:pb + 64, k * 64:(k + 1) * 64] = lo[k]
        ball[0:64, ji] = b
        ball[64:128, ji] = b
    return wall, ball


def _prep_dec(dec_w, dec_b):
    dwall = np.zeros((128, T * 6), np.float16)
    dball = np.zeros((128, T), np.float32)
    for t in range(T):
        w = dec_w[t, :, :, 0, 0].astype(np.float32).T  # [cin, outc]
        hi, lo = _split16(w)
        for pb in (0, 64):
            dwall[pb:pb + 64, t * 6:t * 6 + 3] = hi
            dwall[pb:pb + 64, t * 6 + 3:t * 6 + 6] = lo
        dball[0:3, t] = dec_b[t]
        dball[32:35, t] = dec_b[t]
    return dwall, dball


# ---------------------------------------------------------------- execution
def _get_exec(plan):
    """Compile (once) and return a callable(in_maps) -> list[dict]."""
    key = repr(plan)
    if key in _PROG_CACHE:
        return _PROG_CACHE[key]
    nc = _build_program(plan)

    import jax
    from jax.sharding import Mesh, PartitionSpec
    from jax.experimental.shard_map import shard_map

    bass2jax.install_neuronx_cc_hook()
    in_names, out_names, out_avals, zero_outs = [], [], [], []
    for alloc in nc.m.functions[0].allocations:
        if not isinstance(alloc, mybir.MemoryLocationSet):
            continue
        name = alloc.memorylocations[0].name
        if alloc.kind == "ExternalInput":
            in_names.append(name)
        elif alloc.kind == "ExternalOutput":
            shape = tuple(alloc.tensor_shape)
            dtype = mybir.dt.np(alloc.dtype)
            out_names.append(name)
            out_avals.append(jax.core.ShapedArray(shape, dtype))
            zero_outs.append(np.zeros(shape, dtype))
    n_params = len(in_names)
    n_outs = len(out_avals)
    all_names = in_names + out_names

    def _body(*args):
        outs = bass2jax._bass_exec_p.bind(
            *args, out_avals=tuple(out_avals), in_names=tuple(all_names),
            out_names=tuple(out_names), lowering_input_output_aliases=(),
            sim_require_finite=True, sim_require_nnan=True, nc=nc)
        return tuple(outs)

    devices = jax.devices()[:NCORES]
    mesh = Mesh(np.asarray(devices), ("core",))
    in_specs = (PartitionSpec("core"),) * (n_params + n_outs)
    out_specs = (PartitionSpec("core"),) * n_outs
    donate = tuple(range(n_params, n_params + n_outs))
    sharded = jax.jit(shard_map(_body, mesh=mesh, in_specs=in_specs,
                                out_specs=out_specs, check_rep=False),
                      donate_argnums=donate, keep_unused=True)

    def run(in_maps):
        concat_in = [np.concatenate([np.asarray(in_maps[c][n])
                                     for c in range(NCORES)], axis=0)
                     for n in in_names]
        concat_zeros = [np.concatenate([z] * NCORES, axis=0) for z in zero_outs]
        outs = sharded(*concat_in, *concat_zeros)
        res = []
        for c in range(NCORES):
            d = {}
            for i, n in enumerate(out_names):
                arr = np.asarray(outs[i])
                per = arr.shape[0] // NCORES
                d[n] = arr[c * per:(c + 1) * per]
            res.append(d)
        return res

    _PROG_CACHE[key] = run
    return run


def kernel(x, alpha0, alphas, g0, gs, enc_w, enc_b, dec_w, dec_b):
    x = np.asarray(x, np.float32)
    sels, sts = _routing(np.asarray(alpha0), np.asarray(alphas),
                         np.asarray(g0), np.asarray(gs))
    plan = _build_plan(sels, sts)
    steps, n_bufs = plan
    jobs = [s for s in steps if s[0] == "conv"]
    run = _get_exec(plan)

    wall, ball = _prep_weights(jobs, np.asarray(enc_w), np.asarray(enc_b))
    dwall, dball = _prep_dec(np.asarray(dec_w), np.asarray(dec_b))

    in_maps = []
    for b in range(B):
        xh, xl = _pack_halves(x[b])
        in_maps.append(dict(x_hi=xh, x_lo=xl, wall=wall, ball=ball,
                            dwall=dwall, dball=dball))
    res = run(in_maps)

    out = np.zeros((T, B, OUT_C, H, W), np.float32)
    for b in range(B):
        out[:, b] = res[b]["y"].reshape(T, OUT_C, H, W)
    return out


# revision 11
# speedup vs baseline: 1.0740x; 1.0740x over previous
"""TRN2 Bass kernel for nn_AsymSearch (gumbel-routed conv chains).

Strategy:
 - Routing (gumbel top-1 per task per layer) depends only on tiny
   alpha/gumbel tensors -> computed on host in numpy, mirroring the
   reference ops. Straight-through scale folds into conv weights.
 - Data-parallel over batch B=8 across 8 NeuronCores; each core runs the
   full task tree for one image. Shared routing prefixes are deduped.
 - Each 3x3 conv runs on the tensor engine as 9 shifted 1x1 convs
   (K=64, M=64, N=512) accumulated in PSUM, with 4-quadrant
   tile_position concurrency (image split into y-halves across SBUF
   partition halves).
 - Precision: fp16 hi/lo 3-pass matmuls (w_hi*h_hi + w_lo*h_hi +
   w_hi*h_lo) -> ~1e-5 relative error end-to-end, required because the
   'normal' task normalization amplifies upstream error ~20x.
 - Decoder 1x1 convs on device; 'normal' task L2-normalization on device
   (ones-matmul partition broadcast + ACT sqrt + DVE reciprocal).
"""
import os
from contextlib import ExitStack

import numpy as np

import concourse.bass as bass
import concourse.tile as tile
from concourse import bacc, mybir
from concourse import bass2jax

# ---------------------------------------------------------------- geometry
T, L, M = 3, 4, 3
B, C, H, W = 8, 64, 128, 128
OUT_C = 3
TAU = 1.0
NCORES = 8

WP = W + 2           # padded row width
SLOTS = 66           # rows per half-buffer: pad/halo + 64 + halo/pad
FREE = SLOTS * WP    # 8580 elements per partition
RPB = 4              # image rows per 512-px block
NBLK = 64 // RPB     # 16 blocks per half
HWPIX = H * W

F32 = mybir.dt.float32
F32R = mybir.dt.float32r
F16 = mybir.dt.float16
AF = mybir.ActivationFunctionType

NORM_TASK = 2  # TASKS = ["semantic", "depth", "normal"]; 'normal' in name

_PROG_CACHE = {}


# ---------------------------------------------------------------- routing
def _routing(alpha0, alphas, g0, gs):
    """Mirror of reference gumbel top-1 routing, numpy float32."""
    sels = np.zeros((T, L), np.int32)
    sts = np.zeros((T, L), np.float32)
    for t in range(T):
        idx = 0
        for l in range(L):
            a = (alpha0[t, 0] if l == 0 else alphas[l - 1, t][idx]).astype(np.float32)
            g = (g0[t, 0] if l == 0 else gs[l - 1, t][idx]).astype(np.float32)
            mx = np.max(a)
            lse = (np.log(np.sum(np.exp(a - mx), dtype=np.float32)) + mx).astype(np.float32)
            logits = ((a - lse) + g) / np.float32(TAU)
            e = np.exp(logits - np.max(logits))
            probs = (e / np.sum(e, dtype=np.float32)).astype(np.float32)
            ni = int(np.argmax(probs))
            p = probs[ni]
            sels[t, l] = ni
            sts[t, l] = np.float32(1.0) - p + p
            idx = ni
    return sels, sts


def _build_plan(sels, sts):
    """Prefix-dedup the task conv chains into a schedule with buffer reuse.

    Returns (steps, n_bufs). steps is a list of either
    ("conv", layer, module, st, in_buf, out_buf) or ("dec", task, buf).
    Buffer ids index a small pool of persistent hi/lo SBUF buffer pairs;
    buffer 0 initially holds the input x. A DFS over the dedup prefix
    tree frees each buffer once its last consumer has been emitted.
    """
    # build prefix tree: node -> children {(l, m, st_hex): node}
    children = {(): {}}
    leaf_tasks = {}
    for t in range(T):
        prefix = ()
        for l in range(L):
            key = (l, int(sels[t, l]), float(sts[t, l]).hex())
            nxt = prefix + (key,)
            children.setdefault(prefix, {})
            if nxt not in children[prefix].values():
                pass
            children[prefix][key] = nxt
            children.setdefault(nxt, {})
            prefix = nxt
        leaf_tasks.setdefault(prefix, []).append(t)

    steps = []
    free = []
    n_bufs = [1]

    def alloc():
        if free:
            return free.pop()
        b = n_bufs[0]
        n_bufs[0] += 1
        return b

    def dfs(node, buf):
        # a buffer is freed right after its last consumer is emitted; Tile
        # WAR-serializes any later overwrite against pending readers.
        for t in leaf_tasks.get(node, []):
            steps.append(("dec", t, buf))
        kids = list(children.get(node, {}).items())
        for i, ((l, m, st_hex), child) in enumerate(kids):
            ob = alloc()
            steps.append(("conv", l, m, float.fromhex(st_hex), buf, ob))
            if i == len(kids) - 1:
                free.append(buf)
            dfs(child, ob)
        if not kids:
            free.append(buf)

    dfs((), 0)
    return steps, n_bufs[0]


# ---------------------------------------------------------------- device program
def _emit_conv(nc, psum_pool, tmp_pool, hin_hi, hin_lo, hout_hi, hout_lo,
               w_hi, w_lo, btile):
    """3x3 conv + bias + relu, fp16 3-pass, 4-quadrant scheme."""
    n_mm = int(os.environ.get("KBENCH_NMM", "27"))   # timing experiments only
    force_diag = os.environ.get("KBENCH_DIAG", "0") == "1"
    ihi = hin_hi.rearrange("p (s w) -> p s w", w=WP)
    ilo = hin_lo.rearrange("p (s w) -> p s w", w=WP)
    ohi = hout_hi.rearrange("p (s w) -> p s w", w=WP)
    olo = hout_lo.rearrange("p (s w) -> p s w", w=WP)
    taps = [(dy, dx) for dy in (-1, 0, 1) for dx in (-1, 0, 1)]

    # process blocks in groups of 2 (one diagonal + one off-diagonal psum
    # pair) so FOUR quadrant matmul streams are in flight -> full PE array.
    for grp in range(NBLK // 2):
        blkA, blkB = 2 * grp, 2 * grp + 1
        yA, yB = blkA * RPB, blkB * RPB
        psA = psum_pool.tile([128, 512], F32, tag="convpsA", name="psA", bufs=3)
        psB = psum_pool.tile([128, 512], F32, tag="convpsB", name="psB", bufs=3)
        mm_i = 0
        for k, (dy, dx) in enumerate(taps):
            for pi, (wt, hv) in enumerate([(w_hi, ihi), (w_lo, ihi), (w_hi, ilo)]):
                mm_i += 1
                if mm_i > n_mm:
                    continue
                st = (k == 0 and pi == 0)
                sp = (mm_i == n_mm) or (k == 8 and pi == 2)
                for half in (0, 1):
                    pb = 64 * half
                    lhsT = wt[pb:pb + 64, k * 64:(k + 1) * 64]
                    # pair A diagonal: top->(r0,c0), bottom->(r1,c1)
                    rhsA = hv[pb:pb + 64, yA + 1 + dy:yA + 1 + dy + RPB,
                              1 + dx:1 + dx + W]
                    nc.tensor.matmul(psA[pb:pb + 64, :], lhsT, rhsA,
                                     start=st, stop=sp, tile_position=(pb, pb))
                    # pair B off-diagonal: top->(r0,c1), bottom->(r1,c0)
                    cb = 64 - pb
                    rhsB = hv[pb:pb + 64, yB + 1 + dy:yB + 1 + dy + RPB,
                              1 + dx:1 + dx + W]
                    nc.tensor.matmul(psB[cb:cb + 64, :], lhsT, rhsB,
                                     start=st, stop=sp, tile_position=(pb, cb))
        # epilogues: relu(conv+bias) in fp32, then split hi/lo fp16
        epi = os.environ.get("KBENCH_EPI", "full")  # full|nolo|min
        for blk, ps, offdiag in ((blkA, psA, False), (blkB, psB, True)):
            y0 = blk * RPB
            full = tmp_pool.tile([128, 512], F32, tag="full", name="full")
            if epi == "min":
                nc.vector.tensor_copy(full[:, :], ps[:, :])
                continue
            if not offdiag:
                nc.scalar.activation(full[:, :], ps[:, :], AF.Relu,
                                     bias=btile[:, 0:1])
            else:
                nc.scalar.activation(full[0:64, :], ps[64:128, :], AF.Relu,
                                     bias=btile[0:64, 0:1])
                nc.scalar.activation(full[64:128, :], ps[0:64, :], AF.Relu,
                                     bias=btile[64:128, 0:1])
            hi_dst = ohi[:, y0 + 1:y0 + 1 + RPB, 1:1 + W]
            lo_dst = olo[:, y0 + 1:y0 + 1 + RPB, 1:1 + W]
            nc.scalar.activation(hi_dst, full[:, :], AF.Copy)
            if epi != "nolo":
                nc.vector.tensor_tensor(lo_dst, full[:, :], hi_dst,
                                        mybir.AluOpType.subtract)
    # halo rows between halves (partition-shifted copies)
    for hv in (ohi, olo):
        nc.scalar.activation(hv[0:64, 65, 1:1 + W], hv[64:128, 1, 1:1 + W], AF.Copy)
        nc.scalar.activation(hv[64:128, 0, 1:1 + W], hv[0:64, 64, 1:1 + W], AF.Copy)


def _emit_decoder(nc, psum_pool, small_pool, h_hi, h_lo,
                  dwt, dbt, ydram, task, do_norm, ones_r):
    """1x1 conv decoder (+ optional channel L2 normalization) + DMA out.

    dwt: [128, 6] fp16 tile (hi cols 0:3, lo cols 3:6, dup on both halves)
    dbt: [128, 1] f32 bias tile (values at partitions 0-2 and 32-34)
    ydram: DRAM [OUT_C, HWPIX] f32 slice for this task.
    """
    ihi = h_hi.rearrange("p (s w) -> p s w", w=WP)
    ilo = h_lo.rearrange("p (s w) -> p s w", w=WP)
    for blk in range(2 * NBLK):
        half = blk % 2
        y0 = (blk // 2) * RPB
        pb = 64 * half
        cb = 32 * half          # c0 for top, c32 for bottom (concurrency)
        pix = (64 * half + y0) * W
        ps = psum_pool.tile([64, 512], F32, tag="decps", name="decps", bufs=1)
        for pi, (wcol, hv) in enumerate([(0, ihi), (3, ihi), (0, ilo)]):
            wsl = dwt[pb:pb + 64, wcol:wcol + 3] if pi != 1 else \
                dwt[pb:pb + 64, 3:6]
            rhs = hv[pb:pb + 64, y0 + 1:y0 + 1 + RPB, 1:1 + W]
            nc.tensor.matmul(ps[cb:cb + 3, :], wsl, rhs,
                             start=pi == 0, stop=pi == 2,
                             tile_position=(pb, cb))
        y_t = small_pool.tile([3, 512], F32, tag="ytile", name="y_t")
        nc.scalar.activation(y_t[:, :], ps[cb:cb + 3, :], AF.Identity,
                             bias=dbt[cb:cb + 3, 0:1])
        if not do_norm:
            nc.sync.dma_start(ydram[:, pix:pix + 512], y_t[:, :])
            continue
        # normal task: y / ||y||_2 over channels
        ysq = small_pool.tile([3, 512], F32R, tag="ysq", name="ysq")
        nc.vector.tensor_tensor(ysq[:, :], y_t[:, :], y_t[:, :],
                                mybir.AluOpType.mult)
        ps2 = psum_pool.tile([3, 512], F32, tag="sumps", name="sumps", bufs=1)
        nc.tensor.matmul(ps2[0:3, :], ones_r[0:3, 0:3], ysq[:, :],
                         start=True, stop=True, tile_position=(0, 0))
        nrm = small_pool.tile([3, 512], F32, tag="nrm", name="nrm")
        nc.scalar.activation(nrm[:, :], ps2[0:3, :], AF.Sqrt)
        inv = small_pool.tile([3, 512], F32, tag="inv", name="inv")
        nc.vector.reciprocal(inv[:, :], nrm[:, :])
        yn = small_pool.tile([3, 512], F32, tag="yn", name="yn")
        nc.vector.tensor_tensor(yn[:, :], y_t[:, :], inv[:, :],
                                mybir.AluOpType.mult)
        nc.sync.dma_start(ydram[:, pix:pix + 512], yn[:, :])


def _build_program(plan):
    steps, n_bufs = plan
    njobs = sum(1 for s in steps if s[0] == "conv")
    nc = bacc.Bacc("TRN2", target_bir_lowering=False, debug=False,
                   num_devices=1, enable_partition_id=False)
    x_hi = nc.dram_tensor("x_hi", [128, FREE], F16, kind="ExternalInput").ap()
    x_lo = nc.dram_tensor("x_lo", [128, FREE], F16, kind="ExternalInput").ap()
    wall = nc.dram_tensor("wall", [njobs, 2, 128, 9 * 64], F16,
                          kind="ExternalInput").ap()
    ball = nc.dram_tensor("ball", [128, njobs], F32, kind="ExternalInput").ap()
    dwall = nc.dram_tensor("dwall", [128, T * 6], F16, kind="ExternalInput").ap()
    dball = nc.dram_tensor("dball", [128, T], F32, kind="ExternalInput").ap()
    y = nc.dram_tensor("y", [T, OUT_C, HWPIX], F32, kind="ExternalOutput").ap()

    with tile.TileContext(nc) as tc, ExitStack() as ctx:
        hpool = ctx.enter_context(tc.tile_pool(name="hbufs", bufs=1))
        wpool = ctx.enter_context(tc.tile_pool(name="wpool", bufs=3))
        misc = ctx.enter_context(tc.tile_pool(name="misc", bufs=1))
        tmp_pool = ctx.enter_context(tc.tile_pool(name="tmp", bufs=4))
        small_pool = ctx.enter_context(tc.tile_pool(name="small", bufs=4))
        psum_pool = ctx.enter_context(tc.tile_pool(name="psum", bufs=1,
                                                   space="PSUM"))

        # persistent feature buffers (hi/lo fp16), buffer 0 starts as x
        bufs = []
        for i in range(n_bufs):
            bhi = hpool.tile([128, FREE], F16, name=f"h{i}hi")
            blo = hpool.tile([128, FREE], F16, name=f"h{i}lo")
            bufs.append((bhi, blo))
            if i == 0:
                nc.sync.dma_start(bhi[:, :], x_hi[:, :])
                nc.sync.dma_start(blo[:, :], x_lo[:, :])
            else:
                nc.vector.memset(bhi[:, :], 0.0)
                nc.vector.memset(blo[:, :], 0.0)

        bt_all = misc.tile([128, njobs], F32, name="bt_all")
        nc.sync.dma_start(bt_all[:, :], ball[:, :])
        dbt_all = misc.tile([128, T], F32, name="dbt_all")
        nc.sync.dma_start(dbt_all[:, :], dball[:, :])
        dwt_all = misc.tile([128, T * 6], F16, name="dwt_all")
        nc.sync.dma_start(dwt_all[:, :], dwall[:, :])
        ones_f = misc.tile([3, 3], F32, name="ones_f")
        nc.vector.memset(ones_f[:, :], 1.0)
        ones_r = misc.tile([3, 3], F32R, name="ones_r")
        nc.vector.tensor_copy(ones_r[:, :], ones_f[:, :])

        ji = 0
        for step in steps:
            if step[0] == "conv":
                _, layer, module, st, in_b, out_b = step
                w_hi = wpool.tile([128, 9 * 64], F16, tag="whi", name="w_hi")
                w_lo = wpool.tile([128, 9 * 64], F16, tag="wlo", name="w_lo")
                nc.sync.dma_start(w_hi[:, :], wall[ji, 0, :, :])
                nc.sync.dma_start(w_lo[:, :], wall[ji, 1, :, :])
                _emit_conv(nc, psum_pool, tmp_pool,
                           bufs[in_b][0], bufs[in_b][1],
                           bufs[out_b][0], bufs[out_b][1],
                           w_hi, w_lo, bt_all[:, ji:ji + 1])
                ji += 1
            else:
                _, t, fb = step
                _emit_decoder(nc, psum_pool, small_pool,
                              bufs[fb][0], bufs[fb][1],
                              dwt_all[:, t * 6:(t + 1) * 6],
                              dbt_all[:, t:t + 1],
                              y[t], t, t == NORM_TASK, ones_r)
    nc.compile()
    return nc


# ---------------------------------------------------------------- host packing
def _pack_halves(img):
    """[C, H, W] fp32 -> hi/lo fp16 padded dual-half [128, FREE] arrays."""
    out = []
    hi32 = img.astype(np.float16).astype(np.float32)
    lo = (img - hi32).astype(np.float16)
    hi = img.astype(np.float16)
    for part in (hi, lo):
        buf = np.zeros((128, SLOTS, WP), np.float16)
        buf[0:64, 1:66, 1:129] = part[:, 0:65, :]
        buf[64:128, 0:65, 1:129] = part[:, 63:128, :]
        out.append(buf.reshape(128, FREE))
    return out


def _split16(w):
    hi = w.astype(np.float16)
    lo = (w.astype(np.float32) - hi.astype(np.float32)).astype(np.float16)
    return hi, lo


def _prep_weights(jobs, enc_w, enc_b):
    njobs = len(jobs)
    wall = np.zeros((njobs, 2, 128, 9 * 64), np.float16)
    ball = np.zeros((128, njobs), np.float32)
    for ji, (_, layer, module, st, _, _) in enumerate(jobs):
        w = enc_w[layer, module].astype(np.float32) * np.float32(st)  # OIHW
        b = enc_b[layer, module].astype(np.float32)
        w9 = np.transpose(w, (2, 3, 1, 0)).reshape(9, C, C)  # [tap, cin, cout]
        hi, lo = _split16(w9)
        for k in range(9):
            for pb in (0, 64):
                wall[ji, 0, pb:pb + 64, k * 64:(k + 1) * 64] = hi[k]
                wall[ji, 1, pb:pb + 64, k * 64:(k + 1) * 64] = lo[k]
        ball[0:64, ji] = b
        ball[64:128, ji] = b
    return wall, ball


def _prep_dec(dec_w, dec_b):
    dwall = np.zeros((128, T * 6), np.float16)
    dball = np.zeros((128, T), np.float32)
    for t in range(T):
        w = dec_w[t, :, :, 0, 0].astype(np.float32).T  # [cin, outc]
        hi, lo = _split16(w)
        for pb in (0, 64):
            dwall[pb:pb + 64, t * 6:t * 6 + 3] = hi
            dwall[pb:pb + 64, t * 6 + 3:t * 6 + 6] = lo
        dball[0:3, t] = dec_b[t]
        dball[32:35, t] = dec_b[t]
    return dwall, dball


# ---------------------------------------------------------------- execution
def _get_exec(plan):
    """Compile (once) and return a callable(in_maps) -> list[dict]."""
    key = repr(plan)
    if key in _PROG_CACHE:
        return _PROG_CACHE[key]
    nc = _build_program(plan)

    import jax
    from jax.sharding import Mesh, PartitionSpec
    from jax.experimental.shard_map import shard_map

    bass2jax.install_neuronx_cc_hook()
    in_names, out_names, out_avals, zero_outs = [], [], [], []
    for alloc in nc.m.functions[0].allocations:
        if not isinstance(alloc, mybir.MemoryLocationSet):
            continue
        name = alloc.memorylocations[0].name
        if alloc.kind == "ExternalInput":
            in_names.append(name)
        elif alloc.kind == "ExternalOutput":
            shape = tuple(alloc.tensor_shape)
            dtype = mybir.dt.np(alloc.dtype)
            out_names.append(name)
            out_avals.append(jax.core.ShapedArray(shape, dtype))
            zero_outs.append(np.zeros(shape, dtype))
    n_params = len(in_names)
    n_outs = len(out_avals)
    all_names = in_names + out_names

    def _body(*args):
        outs = bass2jax._bass_exec_p.bind(
            *args, out_avals=tuple(out_avals), in_names=tuple(all_names),
            out_names=tuple(out_names), lowering_input_output_aliases=(),
            sim_require_finite=True, sim_require_nnan=True, nc=nc)
        return tuple(outs)

    devices = jax.devices()[:NCORES]
    mesh = Mesh(np.asarray(devices), ("core",))
    in_specs = (PartitionSpec("core"),) * (n_params + n_outs)
    out_specs = (PartitionSpec("core"),) * n_outs
    donate = tuple(range(n_params, n_params + n_outs))
    sharded = jax.jit(shard_map(_body, mesh=mesh, in_specs=in_specs,
                                out_specs=out_specs, check_rep=False),
                      donate_argnums=donate, keep_unused=True)

    def run(in_maps):
        concat_in = [np.concatenate([np.asarray(in_maps[c][n])
                                     for c in range(NCORES)], axis=0)
                     for n in in_names]
        concat_zeros = [np.concatenate([z] * NCORES, axis=0) for z in zero_outs]
        outs = sharded(*concat_in, *concat_zeros)
        res = []
        for c in range(NCORES):
            d = {}
            for i, n in enumerate(out_names):
                arr = np.asarray(outs[i])
                per = arr.shape[0] // NCORES
                d[n] = arr[c * per:(c + 1) * per]
            res.append(d)
        return res

    _PROG_CACHE[key] = run
    return run


def kernel(x, alpha0, alphas, g0, gs, enc_w, enc_b, dec_w, dec_b):
    x = np.asarray(x, np.float32)
    sels, sts = _routing(np.asarray(alpha0), np.asarray(alphas),
                         np.asarray(g0), np.asarray(gs))
    plan = _build_plan(sels, sts)
    steps, n_bufs = plan
    jobs = [s for s in steps if s[0] == "conv"]
    run = _get_exec(plan)

    wall, ball = _prep_weights(jobs, np.asarray(enc_w), np.asarray(enc_b))
    dwall, dball = _prep_dec(np.asarray(dec_w), np.asarray(dec_b))

    in_maps = []
    for b in range(B):
        xh, xl = _pack_halves(x[b])
        in_maps.append(dict(x_hi=xh, x_lo=xl, wall=wall, ball=ball,
                            dwall=dwall, dball=dball))
    res = run(in_maps)

    out = np.zeros((T, B, OUT_C, H, W), np.float32)
    for b in range(B):
        out[:, b] = res[b]["y"].reshape(T, OUT_C, H, W)
    return out


# revision 12
# speedup vs baseline: 2697.5468x; 2511.7102x over previous
"""TRN2 Bass kernel for nn_AsymSearch (gumbel-routed conv chains).

Strategy:
 - Routing (gumbel top-1 per task per layer) depends only on tiny
   alpha/gumbel tensors -> computed on host in numpy, mirroring the
   reference ops. Straight-through scale folds into conv weights.
 - Data-parallel over batch B=8 across 8 NeuronCores; each core runs the
   full task tree for one image. Shared routing prefixes are deduped.
 - Each 3x3 conv runs on the tensor engine as 9 shifted 1x1 convs
   (K=64, M=64, N=512) accumulated in PSUM, with 4-quadrant
   tile_position concurrency (image split into y-halves across SBUF
   partition halves).
 - Precision: fp16 hi/lo 3-pass matmuls (w_hi*h_hi + w_lo*h_hi +
   w_hi*h_lo) -> ~1e-5 relative error end-to-end, required because the
   'normal' task normalization amplifies upstream error ~20x.
 - Decoder 1x1 convs on device; 'normal' task L2-normalization on device
   (ones-matmul partition broadcast + ACT sqrt + DVE reciprocal).
"""
import os
from contextlib import ExitStack

import numpy as np

import concourse.bass as bass
import concourse.tile as tile
from concourse import bacc, mybir
from concourse import bass2jax

# ---------------------------------------------------------------- geometry
T, L, M = 3, 4, 3
B, C, H, W = 8, 64, 128, 128
OUT_C = 3
TAU = 1.0
NCORES = 8

WP = W + 2           # padded row width
SLOTS = 66           # rows per half-buffer: pad/halo + 64 + halo/pad
FREE = SLOTS * WP    # 8580 elements per partition
RPB = 4              # image rows per 512-px block
NBLK = 64 // RPB     # 16 blocks per half
HWPIX = H * W

F32 = mybir.dt.float32
F32R = mybir.dt.float32r
F16 = mybir.dt.float16
AF = mybir.ActivationFunctionType

NORM_TASK = 2  # TASKS = ["semantic", "depth", "normal"]; 'normal' in name

_PROG_CACHE = {}


# ---------------------------------------------------------------- routing
def _routing(alpha0, alphas, g0, gs):
    """Mirror of reference gumbel top-1 routing, numpy float32."""
    sels = np.zeros((T, L), np.int32)
    sts = np.zeros((T, L), np.float32)
    for t in range(T):
        idx = 0
        for l in range(L):
            a = (alpha0[t, 0] if l == 0 else alphas[l - 1, t][idx]).astype(np.float32)
            g = (g0[t, 0] if l == 0 else gs[l - 1, t][idx]).astype(np.float32)
            mx = np.max(a)
            lse = (np.log(np.sum(np.exp(a - mx), dtype=np.float32)) + mx).astype(np.float32)
            logits = ((a - lse) + g) / np.float32(TAU)
            e = np.exp(logits - np.max(logits))
            probs = (e / np.sum(e, dtype=np.float32)).astype(np.float32)
            ni = int(np.argmax(probs))
            p = probs[ni]
            sels[t, l] = ni
            sts[t, l] = np.float32(1.0) - p + p
            idx = ni
    return sels, sts


def _build_plan(sels, sts):
    """Prefix-dedup the task conv chains into a schedule with buffer reuse.

    Returns (steps, n_bufs). steps is a list of either
    ("conv", layer, module, st, in_buf, out_buf) or ("dec", task, buf).
    Buffer ids index a small pool of persistent hi/lo SBUF buffer pairs;
    buffer 0 initially holds the input x. A DFS over the dedup prefix
    tree frees each buffer once its last consumer has been emitted.
    """
    # build prefix tree: node -> children {(l, m, st_hex): node}
    children = {(): {}}
    leaf_tasks = {}
    for t in range(T):
        prefix = ()
        for l in range(L):
            key = (l, int(sels[t, l]), float(sts[t, l]).hex())
            nxt = prefix + (key,)
            children.setdefault(prefix, {})
            if nxt not in children[prefix].values():
                pass
            children[prefix][key] = nxt
            children.setdefault(nxt, {})
            prefix = nxt
        leaf_tasks.setdefault(prefix, []).append(t)

    steps = []
    free = []
    n_bufs = [1]

    def alloc():
        if free:
            return free.pop()
        b = n_bufs[0]
        n_bufs[0] += 1
        return b

    def dfs(node, buf):
        # a buffer is freed right after its last consumer is emitted; Tile
        # WAR-serializes any later overwrite against pending readers.
        for t in leaf_tasks.get(node, []):
            steps.append(("dec", t, buf))
        kids = list(children.get(node, {}).items())
        for i, ((l, m, st_hex), child) in enumerate(kids):
            ob = alloc()
            steps.append(("conv", l, m, float.fromhex(st_hex), buf, ob))
            if i == len(kids) - 1:
                free.append(buf)
            dfs(child, ob)
        if not kids:
            free.append(buf)

    dfs((), 0)
    return steps, n_bufs[0]


# ---------------------------------------------------------------- device program
def _emit_conv(nc, psum_pool, tmp_pool, hin_hi, hin_lo, hout_hi, hout_lo,
               w_hi, w_lo, btile):
    """3x3 conv + bias + relu, fp16 3-pass, 4-quadrant scheme."""
    ihi = hin_hi.rearrange("p (s w) -> p s w", w=WP)
    ilo = hin_lo.rearrange("p (s w) -> p s w", w=WP)
    ohi = hout_hi.rearrange("p (s w) -> p s w", w=WP)
    olo = hout_lo.rearrange("p (s w) -> p s w", w=WP)
    taps = [(dy, dx) for dy in (-1, 0, 1) for dx in (-1, 0, 1)]

    # process blocks in groups of 2 (one diagonal + one off-diagonal psum
    # pair) so FOUR quadrant matmul streams are in flight -> full PE array.
    for grp in range(NBLK // 2):
        blkA, blkB = 2 * grp, 2 * grp + 1
        yA, yB = blkA * RPB, blkB * RPB
        psA = psum_pool.tile([128, 512], F32, tag="convpsA", name="psA", bufs=3)
        psB = psum_pool.tile([128, 512], F32, tag="convpsB", name="psB", bufs=3)
        for k, (dy, dx) in enumerate(taps):
            for pi, (wt, hv) in enumerate([(w_hi, ihi), (w_lo, ihi), (w_hi, ilo)]):
                st = (k == 0 and pi == 0)
                sp = (k == 8 and pi == 2)
                for half in (0, 1):
                    pb = 64 * half
                    lhsT = wt[pb:pb + 64, k * 64:(k + 1) * 64]
                    # pair A diagonal: top->(r0,c0), bottom->(r1,c1)
                    rhsA = hv[pb:pb + 64, yA + 1 + dy:yA + 1 + dy + RPB,
                              1 + dx:1 + dx + W]
                    nc.tensor.matmul(psA[pb:pb + 64, :], lhsT, rhsA,
                                     start=st, stop=sp, tile_position=(pb, pb))
                    # pair B off-diagonal: top->(r0,c1), bottom->(r1,c0)
                    cb = 64 - pb
                    rhsB = hv[pb:pb + 64, yB + 1 + dy:yB + 1 + dy + RPB,
                              1 + dx:1 + dx + W]
                    nc.tensor.matmul(psB[cb:cb + 64, :], lhsT, rhsB,
                                     start=st, stop=sp, tile_position=(pb, cb))
        # epilogues: relu(conv+bias) in fp32, then split hi/lo fp16
        for blk, ps, offdiag in ((blkA, psA, False), (blkB, psB, True)):
            y0 = blk * RPB
            full = tmp_pool.tile([128, 512], F32, tag="full", name="full")
            if not offdiag:
                nc.scalar.activation(full[:, :], ps[:, :], AF.Relu,
                                     bias=btile[:, 0:1])
            else:
                nc.scalar.activation(full[0:64, :], ps[64:128, :], AF.Relu,
                                     bias=btile[0:64, 0:1])
                nc.scalar.activation(full[64:128, :], ps[0:64, :], AF.Relu,
                                     bias=btile[64:128, 0:1])
            hi_dst = ohi[:, y0 + 1:y0 + 1 + RPB, 1:1 + W]
            lo_dst = olo[:, y0 + 1:y0 + 1 + RPB, 1:1 + W]
            nc.scalar.activation(hi_dst, full[:, :], AF.Copy)
            nc.vector.tensor_tensor(lo_dst, full[:, :], hi_dst,
                                    mybir.AluOpType.subtract)
    # halo rows between halves (partition-shifted copies)
    for hv in (ohi, olo):
        nc.scalar.activation(hv[0:64, 65, 1:1 + W], hv[64:128, 1, 1:1 + W], AF.Copy)
        nc.scalar.activation(hv[64:128, 0, 1:1 + W], hv[0:64, 64, 1:1 + W], AF.Copy)


def _emit_decoder(nc, psum_pool, small_pool, h_hi, h_lo,
                  dwt, dbt, ydram, task, do_norm, ones_r):
    """1x1 conv decoder (+ optional channel L2 normalization) + DMA out.

    dwt: [128, 6] fp16 tile (hi cols 0:3, lo cols 3:6, dup on both halves)
    dbt: [128, 1] f32 bias tile (values at partitions 0-2 and 32-34)
    ydram: DRAM [OUT_C, HWPIX] f32 slice for this task.
    """
    ihi = h_hi.rearrange("p (s w) -> p s w", w=WP)
    ilo = h_lo.rearrange("p (s w) -> p s w", w=WP)
    for blk in range(2 * NBLK):
        half = blk % 2
        y0 = (blk // 2) * RPB
        pb = 64 * half
        cb = 32 * half          # c0 for top, c32 for bottom (concurrency)
        pix = (64 * half + y0) * W
        ps = psum_pool.tile([64, 512], F32, tag="decps", name="decps", bufs=1)
        for pi, (wcol, hv) in enumerate([(0, ihi), (3, ihi), (0, ilo)]):
            wsl = dwt[pb:pb + 64, wcol:wcol + 3] if pi != 1 else \
                dwt[pb:pb + 64, 3:6]
            rhs = hv[pb:pb + 64, y0 + 1:y0 + 1 + RPB, 1:1 + W]
            nc.tensor.matmul(ps[cb:cb + 3, :], wsl, rhs,
                             start=pi == 0, stop=pi == 2,
                             tile_position=(pb, cb))
        y_t = small_pool.tile([3, 512], F32, tag="ytile", name="y_t")
        nc.scalar.activation(y_t[:, :], ps[cb:cb + 3, :], AF.Identity,
                             bias=dbt[cb:cb + 3, 0:1])
        if not do_norm:
            nc.sync.dma_start(ydram[:, pix:pix + 512], y_t[:, :])
            continue
        # normal task: y / ||y||_2 over channels
        ysq = small_pool.tile([3, 512], F32R, tag="ysq", name="ysq")
        nc.vector.tensor_tensor(ysq[:, :], y_t[:, :], y_t[:, :],
                                mybir.AluOpType.mult)
        ps2 = psum_pool.tile([3, 512], F32, tag="sumps", name="sumps", bufs=1)
        nc.tensor.matmul(ps2[0:3, :], ones_r[0:3, 0:3], ysq[:, :],
                         start=True, stop=True, tile_position=(0, 0))
        nrm = small_pool.tile([3, 512], F32, tag="nrm", name="nrm")
        nc.scalar.activation(nrm[:, :], ps2[0:3, :], AF.Sqrt)
        inv = small_pool.tile([3, 512], F32, tag="inv", name="inv")
        nc.vector.reciprocal(inv[:, :], nrm[:, :])
        yn = small_pool.tile([3, 512], F32, tag="yn", name="yn")
        nc.vector.tensor_tensor(yn[:, :], y_t[:, :], inv[:, :],
                                mybir.AluOpType.mult)
        nc.sync.dma_start(ydram[:, pix:pix + 512], yn[:, :])


def _build_program(plan):
    steps, n_bufs = plan
    njobs = sum(1 for s in steps if s[0] == "conv")
    nc = bacc.Bacc("TRN2", target_bir_lowering=False, debug=False,
                   num_devices=1, enable_partition_id=False)
    x_hi = nc.dram_tensor("x_hi", [128, FREE], F16, kind="ExternalInput").ap()
    x_lo = nc.dram_tensor("x_lo", [128, FREE], F16, kind="ExternalInput").ap()
    wall = nc.dram_tensor("wall", [njobs, 2, 128, 9 * 64], F16,
                          kind="ExternalInput").ap()
    ball = nc.dram_tensor("ball", [128, njobs], F32, kind="ExternalInput").ap()
    dwall = nc.dram_tensor("dwall", [128, T * 6], F16, kind="ExternalInput").ap()
    dball = nc.dram_tensor("dball", [128, T], F32, kind="ExternalInput").ap()
    y = nc.dram_tensor("y", [T, OUT_C, HWPIX], F32, kind="ExternalOutput").ap()

    with tile.TileContext(nc) as tc, ExitStack() as ctx:
        hpool = ctx.enter_context(tc.tile_pool(name="hbufs", bufs=1))
        wpool = ctx.enter_context(tc.tile_pool(name="wpool", bufs=3))
        misc = ctx.enter_context(tc.tile_pool(name="misc", bufs=1))
        tmp_pool = ctx.enter_context(tc.tile_pool(name="tmp", bufs=4))
        small_pool = ctx.enter_context(tc.tile_pool(name="small", bufs=4))
        psum_pool = ctx.enter_context(tc.tile_pool(name="psum", bufs=1,
                                                   space="PSUM"))

        # persistent feature buffers (hi/lo fp16), buffer 0 starts as x
        bufs = []
        for i in range(n_bufs):
            bhi = hpool.tile([128, FREE], F16, name=f"h{i}hi")
            blo = hpool.tile([128, FREE], F16, name=f"h{i}lo")
            bufs.append((bhi, blo))
            if i == 0:
                nc.sync.dma_start(bhi[:, :], x_hi[:, :])
                nc.sync.dma_start(blo[:, :], x_lo[:, :])
            else:
                nc.vector.memset(bhi[:, :], 0.0)
                nc.vector.memset(blo[:, :], 0.0)

        bt_all = misc.tile([128, njobs], F32, name="bt_all")
        nc.sync.dma_start(bt_all[:, :], ball[:, :])
        dbt_all = misc.tile([128, T], F32, name="dbt_all")
        nc.sync.dma_start(dbt_all[:, :], dball[:, :])
        dwt_all = misc.tile([128, T * 6], F16, name="dwt_all")
        nc.sync.dma_start(dwt_all[:, :], dwall[:, :])
        ones_f = misc.tile([3, 3], F32, name="ones_f")
        nc.vector.memset(ones_f[:, :], 1.0)
        ones_r = misc.tile([3, 3], F32R, name="ones_r")
        nc.vector.tensor_copy(ones_r[:, :], ones_f[:, :])

        ji = 0
        for step in steps:
            if step[0] == "conv":
                _, layer, module, st, in_b, out_b = step
                w_hi = wpool.tile([128, 9 * 64], F16, tag="whi", name="w_hi")
                w_lo = wpool.tile([128, 9 * 64], F16, tag="wlo", name="w_lo")
                nc.sync.dma_start(w_hi[:, :], wall[ji, 0, :, :])
                nc.sync.dma_start(w_lo[:, :], wall[ji, 1, :, :])
                _emit_conv(nc, psum_pool, tmp_pool,
                           bufs[in_b][0], bufs[in_b][1],
                           bufs[out_b][0], bufs[out_b][1],
                           w_hi, w_lo, bt_all[:, ji:ji + 1])
                ji += 1
            else:
                _, t, fb = step
                _emit_decoder(nc, psum_pool, small_pool,
                              bufs[fb][0], bufs[fb][1],
                              dwt_all[:, t * 6:(t + 1) * 6],
                              dbt_all[:, t:t + 1],
                              y[t], t, t == NORM_TASK, ones_r)
    nc.compile()
    return nc


# ---------------------------------------------------------------- host packing
def _pack_halves(img):
    """[C, H, W] fp32 -> hi/lo fp16 padded dual-half [128, FREE] arrays."""
    out = []
    hi32 = img.astype(np.float16).astype(np.float32)
    lo = (img - hi32).astype(np.float16)
    hi = img.astype(np.float16)
    for part in (hi, lo):
        buf = np.zeros((128, SLOTS, WP), np.float16)
        buf[0:64, 1:66, 1:129] = part[:, 0:65, :]
        buf[64:128, 0:65, 1:129] = part[:, 63:128, :]
        out.append(buf.reshape(128, FREE))
    return out


def _split16(w):
    hi = w.astype(np.float16)
    lo = (w.astype(np.float32) - hi.astype(np.float32)).astype(np.float16)
    return hi, lo


def _prep_weights(jobs, enc_w, enc_b):
    njobs = len(jobs)
    wall = np.zeros((njobs, 2, 128, 9 * 64), np.float16)
    ball = np.zeros((128, njobs), np.float32)
    for ji, (_, layer, module, st, _, _) in enumerate(jobs):
        w = enc_w[layer, module].astype(np.float32) * np.float32(st)  # OIHW
        b = enc_b[layer, module].astype(np.float32)
        w9 = np.transpose(w, (2, 3, 1, 0)).reshape(9, C, C)  # [tap, cin, cout]
        hi, lo = _split16(w9)
        for k in range(9):
            for pb in (0, 64):
                wall[ji, 0, pb:pb + 64, k * 64:(k + 1) * 64] = hi[k]
                wall[ji, 1, pb:pb + 64, k * 64:(k + 1) * 64] = lo[k]
        ball[0:64, ji] = b
        ball[64:128, ji] = b
    return wall, ball


def _prep_dec(dec_w, dec_b):
    dwall = np.zeros((128, T * 6), np.float16)
    dball = np.zeros((128, T), np.float32)
    for t in range(T):
        w = dec_w[t, :, :, 0, 0].astype(np.float32).T  # [cin, outc]
        hi, lo = _split16(w)
        for pb in (0, 64):
            dwall[pb:pb + 64, t * 6:t * 6 + 3] = hi
            dwall[pb:pb + 64, t * 6 + 3:t * 6 + 6] = lo
        dball[0:3, t] = dec_b[t]
        dball[32:35, t] = dec_b[t]
    return dwall, dball


# ---------------------------------------------------------------- execution
def _get_exec(plan):
    """Compile (once) and return a callable(in_maps) -> list[dict]."""
    key = repr(plan)
    if key in _PROG_CACHE:
        return _PROG_CACHE[key]
    nc = _build_program(plan)

    import jax
    from jax.sharding import Mesh, PartitionSpec
    from jax.experimental.shard_map import shard_map

    bass2jax.install_neuronx_cc_hook()
    in_names, out_names, out_avals, zero_outs = [], [], [], []
    for alloc in nc.m.functions[0].allocations:
        if not isinstance(alloc, mybir.MemoryLocationSet):
            continue
        name = alloc.memorylocations[0].name
        if alloc.kind == "ExternalInput":
            in_names.append(name)
        elif alloc.kind == "ExternalOutput":
            shape = tuple(alloc.tensor_shape)
            dtype = mybir.dt.np(alloc.dtype)
            out_names.append(name)
            out_avals.append(jax.core.ShapedArray(shape, dtype))
            zero_outs.append(np.zeros(shape, dtype))
    n_params = len(in_names)
    n_outs = len(out_avals)
    all_names = in_names + out_names

    def _body(*args):
        outs = bass2jax._bass_exec_p.bind(
            *args, out_avals=tuple(out_avals), in_names=tuple(all_names),
            out_names=tuple(out_names), lowering_input_output_aliases=(),
            sim_require_finite=True, sim_require_nnan=True, nc=nc)
        return tuple(outs)

    devices = jax.devices()[:NCORES]
    mesh = Mesh(np.asarray(devices), ("core",))
    in_specs = (PartitionSpec("core"),) * (n_params + n_outs)
    out_specs = (PartitionSpec("core"),) * n_outs
    donate = tuple(range(n_params, n_params + n_outs))
    sharded = jax.jit(shard_map(_body, mesh=mesh, in_specs=in_specs,
                                out_specs=out_specs, check_rep=False),
                      donate_argnums=donate, keep_unused=True)

    def run(in_maps):
        concat_in = [np.concatenate([np.asarray(in_maps[c][n])
                                     for c in range(NCORES)], axis=0)
                     for n in in_names]
        concat_zeros = [np.concatenate([z] * NCORES, axis=0) for z in zero_outs]
        outs = sharded(*concat_in, *concat_zeros)
        res = []
        for c in range(NCORES):
            d = {}
            for i, n in enumerate(out_names):
                arr = np.asarray(outs[i])
                per = arr.shape[0] // NCORES
                d[n] = arr[c * per:(c + 1) * per]
            res.append(d)
        return res

    _PROG_CACHE[key] = run
    return run


def kernel(x, alpha0, alphas, g0, gs, enc_w, enc_b, dec_w, dec_b):
    x = np.asarray(x, np.float32)
    sels, sts = _routing(np.asarray(alpha0), np.asarray(alphas),
                         np.asarray(g0), np.asarray(gs))
    plan = _build_plan(sels, sts)
    steps, n_bufs = plan
    jobs = [s for s in steps if s[0] == "conv"]
    run = _get_exec(plan)

    wall, ball = _prep_weights(jobs, np.asarray(enc_w), np.asarray(enc_b))
    dwall, dball = _prep_dec(np.asarray(dec_w), np.asarray(dec_b))

    in_maps = []
    for b in range(B):
        xh, xl = _pack_halves(x[b])
        in_maps.append(dict(x_hi=xh, x_lo=xl, wall=wall, ball=ball,
                            dwall=dwall, dball=dball))
    res = run(in_maps)

    out = np.zeros((T, B, OUT_C, H, W), np.float32)
    for b in range(B):
        out[:, b] = res[b]["y"].reshape(T, OUT_C, H, W)
    return out
